# revision 18
# baseline (speedup 1.0000x reference)
"""HDC binary attention kernel for 8 trn2 NeuronCores.

Problem: B,T,D = 4,2048,1024
    Q = sign(x * sign(bv_q)); K = sign(x * sign(bv_k)); V = x * sign(bv_v)
    scores = (Q @ K^T) / sqrt(D), causal
    out = sigmoid(4*scores) * causal_mask @ V

Math used by the kernel:
    sign(x*bq) = sign(x)*sign(bq) elementwise, so with S = sign(x) (+-1) and
    c[d] = sign(bv_q)[d]*sign(bv_k)[d]:
        scores[t,s] = sum_d S[t,d]*c[d]*S[s,d] / 32
    We compute scores TRANSPOSED (s on partitions) via
        scoresT = SkT.T @ (c * SqT)   (contraction d on partitions)
    then attnT = sigmoid(scoresT * 0.125) (* mask on diagonal chunks), fp16,
    and out = attnT.T @ V accumulated over s-subtiles (fp16 matmul).

Precision/layout choices:
  - Host supplies x in BOTH layouts as bf16 (sign(bf16(x)) == sign(x), and
    bf16 V is well within the error budget): a [dk,s]-tiled transposed
    layout for the S^T path and an s-tiled natural layout for the V path.
    No on-device transposes.
  - S^T and c*S^T are +-1, stored as fp8e4 (exact); the score matmuls run
    in fp8 DoubleRow mode (256-deep contraction per matmul, 0.5 cyc/row).
    Products are +-1 and accumulate in fp32 PSUM, so scores are exact.
  - attn and V are fp16; output fp16 (upcast on host).
  - DMAs are coarse (one per 512-col s-block / 512-row s-group) to bound
    HWDGE descriptor-generation occupancy; the first block is split in 4 so
    the PE can start early.

Sharding: 2 cores per batch. Each 512-row chunk of T is split in half:
    core parity 0 takes rows [512j, 512j+256), parity 1 takes [512j+256, 512j+512).
For SPMD uniformity the host permutes K/V rows for parity-1 cores (swapping the
halves of every 512-chunk) so that each core's q rows always sit at canonical
positions [512j, 512j+256); causal boundary handling is via host-built masks.
Each q group j attends to canonical s < 512*(j+1); full 512-chunks below the
boundary are permutation-invariant, the boundary chunk is masked explicitly.
"""

import numpy as np

B, T, D = 4, 2048, 1024
NQ = 1024          # q rows per core
NCORES = 8
ST = 16            # s-tiles of 128 rows
DT = 8             # d-tiles of 128
NG = 4             # q groups of 256 rows per core

_CACHE = {}


def build_nc():
    """Build + schedule + compile the (single, SPMD-uniform) bass program."""
    import concourse.bass as bass
    import concourse.bacc as bacc
    import concourse.mybir as mybir
    import concourse.tile as tile

    fp32 = mybir.dt.float32
    bf16 = mybir.dt.bfloat16
    fp16 = mybir.dt.float16
    fp8 = mybir.dt.float8e4
    AF = mybir.ActivationFunctionType
    DR = mybir.MatmulPerfMode.DoubleRow

    nc = bacc.Bacc("TRN2", target_bir_lowering=False, debug=False)

    # xtb[q, p, dk, s]: x^T[128dk+p, 512q+s]  (bf16, one DMA per s-block)
    xtb_d = nc.dram_tensor("xtb", [4, 128, DT * 512], bf16, kind="ExternalInput").ap()
    # xnb[i, p, j, d]: x[512i+128j+p, d]      (bf16, one DMA per s-group)
    xnb_d = nc.dram_tensor("xnb", [4, 128, 4 * D], bf16, kind="ExternalInput").ap()
    # cbig[p, dk, j] = c[128dk+p]  (fp8, +-1)
    cbig_d = nc.dram_tensor("cbig", [128, DT * 256], fp8, kind="ExternalInput").ap()
    bvs_d = nc.dram_tensor("bvs", [128, D], bf16, kind="ExternalInput").ap()
    # maskp[p, 256*wq+ct]: keep for boundary s-offset (128*wq+p) vs q col ct
    mask_d = nc.dram_tensor("maskp", [128, 4 * 256], fp16, kind="ExternalInput").ap()
    out_d = nc.dram_tensor("out", [NQ, D], fp16, kind="ExternalOutput").ap()

    with tile.TileContext(nc) as tc:
        with (
            tc.tile_pool(name="const", bufs=1) as constp,
            tc.tile_pool(name="load", bufs=2) as loadp,
            tc.tile_pool(name="vload", bufs=2) as vloadp,
            tc.tile_pool(name="kt", bufs=1) as ktp,
            tc.tile_pool(name="qt", bufs=1) as qtp,
            tc.tile_pool(name="vv", bufs=1) as vvp,
            tc.tile_pool(name="at", bufs=1) as atp,
            tc.tile_pool(name="ps", bufs=3, space="PSUM") as psp,
            tc.tile_pool(name="po", bufs=3, space="PSUM") as pop,
            tc.tile_pool(name="pof", bufs=2, space="PSUM") as pofp,
            tc.tile_pool(name="outb", bufs=3) as outp,
        ):
            # ---- constants ----
            # All input DMAs go on the sync queue: transfers serialize on the
            # shared DMA engines in acquisition order, so a single queue in
            # emission order is the only way to control transfer priority
            # (DMAs on other queues race past stalled compute ops).
            cbig_sb = constp.tile([128, DT, 256], fp8, tag="cbig")
            mask_sb = constp.tile([128, 4 * 256], fp16, tag="maskp")
            bvs_sb = constp.tile([128, D], bf16, tag="bvs")

            # Tiny sigmoid first so the act-table pass loads the combined
            # sign+sigmoid table once, instead of a sign-only table followed
            # by a mid-pipeline reload before the first real sigmoid.
            dumm = constp.tile([128, 1], fp32, tag="dumm")
            dumo = constp.tile([128, 1], fp16, tag="dumo")
            nc.gpsimd.memset(dumm[:], 0.0)
            nc.scalar.activation(dumo[:], dumm[:], AF.Sigmoid)

            def load_cbig():
                nc.sync.dma_start(
                    cbig_sb[:], cbig_d.rearrange("p (dk j) -> p dk j", dk=DT))

            def load_consts2():
                nc.sync.dma_start(mask_sb[:], mask_d)
                nc.sync.dma_start(bvs_sb[:], bvs_d)

            # ---- persistent arrays ----
            # skt8[q]: [128 d-part, dk, 512] fp8 = sign(x)^T for s block q;
            # 3D so DoubleRow matmuls can take [:, 2e:2e+2, cols] slices.
            skt8 = [ktp.tile([128, DT, 512], fp8, tag=f"skt{q}", name=f"skt{q}")
                    for q in range(4)]
            # scq8: [128 d-part, dk, 1024] fp8; [:, dk, 256g+ct] = c*S^T at
            # q col (256g+ct)
            scq8 = qtp.tile([128, DT, NQ], fp8, tag="scq")
            # V[i]: [128 s-part, j, 1024 d] fp16 for s-tiles 4i+j
            vt = [vvp.tile([128, 4, D], fp16, tag=f"v{i}", name=f"v{i}")
                  for i in range(4)]
            # attnT[ss]: [128 s-part, 1024 q] fp16
            att = [atp.tile([128, NQ], fp16, tag=f"att{ss}", name=f"att{ss}")
                   for ss in range(ST)]

            xtts = {}
            xnts = {}

            def load_block_dma(q):
                """4 chunked DMAs of s-block q (2 d-tiles each)."""
                xtt = loadp.tile([128, DT, 512], bf16, tag="xtt", name=f"xtt{q}")
                xtts[q] = xtt
                for e in range(4):
                    nc.sync.dma_start(
                        xtt[:, 2 * e:2 * e + 2, :],
                        xtb_d[q][:, 2 * e * 512:(2 * e + 2) * 512])
                    if q == 0 and e == 0:
                        load_cbig()

            def block_compute(q):
                """Per-chunk sign + scq8 for s-block q."""
                xtt = xtts[q]
                for e in range(4):
                    nc.scalar.activation(skt8[q][:, 2 * e:2 * e + 2, :],
                                         xtt[:, 2 * e:2 * e + 2, :], AF.Sign)
                    # q-cols of group g=q are the first 256 cols of this block
                    nc.vector.tensor_mul(
                        scq8[:, 2 * e:2 * e + 2, q * 256:(q + 1) * 256],
                        skt8[q][:, 2 * e:2 * e + 2, 0:256],
                        cbig_sb[:, 2 * e:2 * e + 2, :])

            def load_vblock_dma(i):
                xnt = vloadp.tile([128, 4, D], bf16, tag="xnt", name=f"xnt{i}")
                xnts[i] = xnt
                nc.sync.dma_start(xnt[:], xnb_d[i])

            def vmuls(i):
                for j in range(4):
                    nc.vector.tensor_mul(vt[i][:, j, :], xnts[i][:, j, :],
                                         bvs_sb[:])

            def score_unit(ss, g):
                """scoresT rows s=[128ss,128ss+128) x q cols [256g, 256g+256)."""
                qb = ss // 4
                wq = ss % 4
                co = wq * 128
                ps = psp.tile([128, 256], fp32, tag="ps", name=f"ps{ss}_{g}")
                for e in range(DT // 2):
                    nc.tensor.matmul(
                        ps[:],
                        skt8[qb][:, 2 * e:2 * e + 2, co:co + 128],
                        scq8[:, 2 * e:2 * e + 2, g * 256:(g + 1) * 256],
                        start=(e == 0),
                        stop=(e == DT // 2 - 1),
                        perf_mode=DR,
                    )
                dst = att[ss][:, g * 256:(g + 1) * 256]
                # attn = sigmoid(scores/32 * 4)
                nc.scalar.activation(dst, ps[:], AF.Sigmoid, scale=0.125)
                if g == qb:
                    # boundary chunk: apply causal mask
                    nc.vector.tensor_mul(dst, dst,
                                         mask_sb[:, wq * 256:(wq + 1) * 256])

            def av(ts, split=False):
                """output rows t=[128ts,128ts+128): accumulate over s prefix.

                split=True runs each 512-col half in two 256-col PSUM
                accumulations so the final copy+DMA chain after the last
                matmul is half as long (used for the last two row-tiles,
                whose chains end the kernel).
                """
                j = ts // 2
                nss = 4 * (j + 1)
                ob = outp.tile([128, D], fp16, tag="ob", name=f"ob{ts}")
                nh = 2 if split else 1
                w = 512 // nh
                for dh in range(2):
                    for h in range(nh):
                        c0 = dh * 512 + h * w
                        if split:
                            po = pofp.tile([128, w], fp32, tag="pof",
                                           name=f"po{ts}_{dh}_{h}")
                        else:
                            po = pop.tile([128, w], fp32, tag="po",
                                          name=f"po{ts}_{dh}")
                        for ss in range(nss):
                            nc.tensor.matmul(
                                po[:],
                                att[ss][:, ts * 128:(ts + 1) * 128],
                                vt[ss // 4][:, ss % 4, c0:c0 + w],
                                start=(ss == 0),
                                stop=(ss == nss - 1),
                            )
                        nc.vector.tensor_copy(ob[:, c0:c0 + w], po[:])
                    q_dma = nc.sync if split else nc.gpsimd
                    q_dma.dma_start(
                        out_d[ts * 128:(ts + 1) * 128,
                              dh * 512:(dh + 1) * 512],
                        ob[:, dh * 512:(dh + 1) * 512])

            # ---- emission order ----
            # Ascending s-blocks. Stage q: load block q (building scq for
            # group g=q), then score all s-tiles ss <= 4q+3 against q-group
            # g=q (their skt blocks are already resident), then the two AV
            # row-tiles of group q, which depend only on those scores. This
            # overlaps AV matmuls with later block DMA instead of
            # serializing all AV at the tail. Block 0 is split into 4
            # DMA/sign chunks so the first scores start early.
            # Sync-queue transfer order: cbig, block0 chunks, xnb0,
            # masks/bvs, then per stage q: xtb(q+1) chunks, xnb(q+1) — so
            # stage-q data always lands before the PE drains stage q-1's
            # work. Each engine executes strictly in order, so per-engine
            # emission must match data-arrival order: the next block's
            # sign/scq (ACT/DVE) are emitted mid-stage, after the current
            # stage's first sigmoids/V-muls.
            load_block_dma(0)
            block_compute(0)
            load_vblock_dma(0)
            load_consts2()
            for q in range(4):
                for ss in range(4 * q + 4):
                    score_unit(ss, q)
                    if ss == 3:
                        vmuls(q)
                        if q < 3:
                            load_block_dma(q + 1)
                            block_compute(q + 1)
                            load_vblock_dma(q + 1)
                av(2 * q, split=(q == 3))
                av(2 * q + 1, split=(q == 3))

    nc.compile()
    return nc


def host_inputs(x, bv_q, bv_k, bv_v):
    """Build per-core input maps (all host work is a cast/copy or O(small))."""
    import ml_dtypes
    bfloat16 = ml_dtypes.bfloat16
    f8 = ml_dtypes.float8_e4m3fn

    x = np.asarray(x, dtype=np.float32)
    sq = np.sign(np.asarray(bv_q, dtype=np.float32))
    sk = np.sign(np.asarray(bv_k, dtype=np.float32))
    sv = np.sign(np.asarray(bv_v, dtype=np.float32))
    c = (sq * sk).astype(np.float32)                     # [D]
    # cbig[p, dk*256+j] = c[128dk+p]
    cbig = np.ascontiguousarray(
        np.broadcast_to(c.reshape(DT, 128).T[:, :, None],
                        (128, DT, 256)).reshape(128, DT * 256)).astype(f8)
    bvs = np.ascontiguousarray(
        np.broadcast_to(sv, (128, D))).astype(bfloat16)

    masks = {}
    for parity in (0, 1):
        wo = np.arange(512)[:, None]                     # boundary s offset
        ct = np.arange(256)[None, :]                     # q col offset in group
        if parity == 0:
            keep = wo <= ct                              # orig offsets equal
        else:
            so = np.where(wo < 256, wo + 256, wo - 256)  # swapped halves
            keep = so <= ct + 256
        # [wq*128+p, ct] -> [p, wq*256+ct]
        m = keep.astype(np.float16).reshape(4, 128, 256)
        masks[parity] = np.ascontiguousarray(
            m.transpose(1, 0, 2).reshape(128, 4 * 256))

    in_maps = []
    for core in range(NCORES):
        b, parity = core // 2, core % 2
        xb = x[b]
        if parity == 0:
            xkc = xb
        else:
            xkc = xb.reshape(NG, 2, 256, D)[:, ::-1].reshape(T, D)
        # xtb[q, p, dk, s] = xkc[512q+s, 128dk+p]
        xtb = np.ascontiguousarray(
            xkc.reshape(4, 512, DT, 128).transpose(0, 3, 2, 1)
        ).astype(bfloat16).reshape(4, 128, DT * 512)
        # xnb[i, p, j, d] = xkc[512i+128j+p, d]
        xnb = np.ascontiguousarray(
            xkc.reshape(4, 4, 128, D).transpose(0, 2, 1, 3)
        ).astype(bfloat16).reshape(4, 128, 4 * D)
        in_maps.append({
            "xtb": xtb,
            "xnb": xnb,
            "cbig": cbig,
            "bvs": bvs,
            "maskp": masks[parity],
        })
    return in_maps


def assemble_output(results):
    out = np.zeros((B, T, D), np.float32)
    for core in range(NCORES):
        b, parity = core // 2, core % 2
        o = np.asarray(results[core]["out"], dtype=np.float32).reshape(NG, 256, D)
        for j in range(NG):
            r0 = 512 * j + 256 * parity
            out[b, r0:r0 + 256] = o[j]
    return out


def kernel(x, bv_q, bv_k, bv_v):
    from concourse.bass_utils import run_bass_kernel_spmd

    if "nc" not in _CACHE:
        _CACHE["nc"] = build_nc()
    nc = _CACHE["nc"]

    in_maps = host_inputs(x, bv_q, bv_k, bv_v)
    res = run_bass_kernel_spmd(nc, in_maps, list(range(NCORES)))
    _CACHE["last_result"] = res
    return assemble_output(res.results)
